# revision 12
# baseline (speedup 1.0000x reference)
"""Trainium2 Bass kernel for nn_DeChunkLayerReference.

The reference collapses mathematically: with state dim n=1, C==1, B=p and
per-(b,t) scalars shared across all heads, the SSD is a per-channel scalar
EMA along the M=2048 compressed sequence:

    y[b,t,:] = exp(-dt[t]) * y[b,t-1,:] + (p[t]/dt[t]) * hidden[b,t,:]

followed by a gather that duplicates each compressed row to the L=4096
output positions (plug = cumsum(boundary_mask)-1).

Closed form: y[t] = sum_{s<=t} exp(cumA[t]-cumA[s]) * w[s] * hidden[s]
with cumA = cumsum(-dt), w = p/dt.  Since dt ~ Exp(1), the decay kernel
underflows fp32 after a couple hundred steps, so y is computed with
chunked (128) lower-triangular matmuls over a few bands of chunks:

    LT_block[s,t] = exp( (cumA[t]-cumA[T0_i]) + (cumA[T0_i]-cumA[s]+log w[s]) )
    y_chunk_i     = sum_bands LT_block(j,i).T @ hidden_chunk_j      (PSUM acc)

The number of bands per chunk is decided on the host from the actual cumA
(a band is included iff its largest coefficient is above the fp32 denormal
floor), so the truncation is exact in fp32.  All per-position scalars are
precomputed on the host in float64 (they depend only on the tiny
boundary_prob/boundary_mask inputs); the exp itself runs on the ACT engine
with the per-partition bias folding in both -cumA[s] and log w[s].

Sharding over the 8 cores: (batch b in {0,1}) x (d_model quarter q in
{0..3}); each core processes its full sequence for a 512-wide channel
slice, so there is no cross-core communication at all.
"""

import numpy as np

import concourse.bass as bass
import concourse.tile as tile
from concourse import bacc, mybir
from concourse.bass_utils import run_bass_kernel_spmd

# Problem shapes (hardcoded per harness contract).
B = 2
M = 2048
D_MODEL = 2048
LFULL = 4096
CHUNK = 128
C = M // CHUNK          # 16 chunks
NCORES = 8
NQ = 4                  # d_model quarters
QW = D_MODEL // NQ      # 512 channels per core
EPS = 1e-4
MNEG = -30000.0         # pre-exp mask for the upper triangle (s > t)
UFLOW = -103.0          # ln(smallest fp32 denormal) ~ -103.28
USE_BF16 = True         # bf16 data path: 4x PE throughput + half DMA bytes

F32 = mybir.dt.float32
BF16 = mybir.dt.bfloat16

_prog_cache: dict = {}


def _host_precompute(boundary_mask, boundary_prob):
    """float64 coefficient prep from the small inputs."""
    bm = np.asarray(boundary_mask)
    bp = np.asarray(boundary_prob)
    p = np.clip(bp[..., -1].astype(np.float32), EPS, 1.0 - EPS)
    token_idx = np.arange(bm.shape[1])[None, :] + (~bm).astype(np.int32) * bm.shape[1]
    order = np.argsort(token_idx, axis=1, kind="stable")
    p_sel = np.take_along_axis(p, order[:, :M], axis=1).astype(np.float64)  # (B, M)
    dt = -np.log1p(-p_sel)
    w = p_sel / dt
    logw = np.log(w)
    cumA = np.cumsum(-dt, axis=1)                       # (B, M) inclusive
    plug = np.cumsum(bm.astype(np.int64), axis=1) - 1   # (B, L)
    return logw, cumA, plug


def _decide_bands(cumA, logw):
    """Bands per chunk (union over batches so the SPMD program is shared)."""
    nb = []
    for i in range(C):
        T0 = i * CHUNK
        n = 1
        for bandk in range(1, i + 1):
            S0 = (i - bandk) * CHUNK
            mx = max(
                (cumA[b, T0] - cumA[b, S0:S0 + CHUNK] + logw[b, S0:S0 + CHUNK]).max()
                for b in range(cumA.shape[0])
            )
            if mx > UFLOW:
                n = bandk + 1
            else:
                break
        nb.append(n)
    return tuple(nb)


# Constants tensor "ct" (128, 128 + maxband*C):
#   [:, 0:128]         mneg — MNEG above the diagonal (s > t), 0 elsewhere
#   [:, 128 + k*C + i] bias column for band k, output chunk i
CT_MNEG = 0
CT_BIAS = CHUNK

# rrow tensor (1, 128 + C*CHUNK) float32r:
#   [0, 0:128]              = 1.0 (ones row, stationary for the PE broadcast)
#   [0, 128 + i*128 : ...]  = R_i[t] = cumA[T0_i + t] - cumA[T0_i]
RR_ONES = 0
RR_R = CHUNK

BCG = 4                        # chunks of R per PE-broadcast matmul (1 bank)


def _build_program(nbands, rep, use_bf16=True):
    maxband = max(nbands)
    ct_w = CHUNK + maxband * C
    nc = bacc.Bacc(
        "TRN2", target_bir_lowering=False, debug=False, num_devices=NCORES
    )
    mm_dt = BF16 if use_bf16 else F32
    F32R = mybir.dt.float32r
    x = nc.dram_tensor("x", [M, QW], mm_dt, kind="ExternalInput")
    rrow = nc.dram_tensor("rrow", [1, CHUNK + C * CHUNK], F32R,
                          kind="ExternalInput")
    ct = nc.dram_tensor("ct", [CHUNK, ct_w], F32, kind="ExternalInput")
    y = nc.dram_tensor("y", [LFULL, QW], mm_dt, kind="ExternalOutput")

    PAIR = 2                     # chunks per output staging tile / DMA
    NBC = C // BCG               # 4 broadcast matmuls

    with tile.TileContext(nc) as tc:
        with tc.tile_pool(name="consts", bufs=1) as consts, \
             tc.tile_pool(name="xp", bufs=1) as xp, \
             tc.tile_pool(name="ltp", bufs=8) as ltp, \
             tc.tile_pool(name="argp", bufs=4) as argp, \
             tc.tile_pool(name="yp", bufs=3) as yp, \
             tc.tile_pool(name="bcp", bufs=1, space="PSUM") as bcp, \
             tc.tile_pool(name="psp", bufs=4, space="PSUM") as psp:

            # Input x: one DMA per chunk pair, issues alternating between
            # the sync and gpsimd queues so transfers start as early as
            # possible and the first pair lands first.
            xin = x.rearrange("(g c p) d -> g p c d", c=2, p=CHUNK)
            xw = []
            for g in range(C // 2):
                t = xp.tile([CHUNK, 2 * QW], mm_dt, tag=f"x{g}")
                eng = nc.sync if g % 2 == 0 else nc.gpsimd
                eng.dma_start(
                    out=t[:].rearrange("p (c d) -> p c d", c=2),
                    in_=xin[g],
                )
                xw.append(t)

            def xview(j):
                g, c = divmod(j, 2)
                return xw[g][:, c * QW:(c + 1) * QW]

            # Small scalar inputs on the scalar queue: rrow feeds the PE
            # broadcast; ct carries the mneg mask and per-band biases.
            rr_sb = consts.tile([1, CHUNK + C * CHUNK], F32R, tag="rr")
            nc.scalar.dma_start(out=rr_sb[:], in_=rrow[:, :])
            ct_sb = consts.tile([CHUNK, ct_w], F32, tag="ct")
            nc.scalar.dma_start(out=ct_sb[:], in_=ct[:, :])
            mneg_v = ct_sb[:, CT_MNEG:CT_MNEG + CHUNK]
            ones_v = rr_sb[:, RR_ONES:RR_ONES + CHUNK]

            # R broadcast: ones[1,128].T @ R[1,512] -> PSUM [128,512].
            # The exps read R straight out of PSUM; no 1 MiB DMA.
            bc = [bcp.tile([CHUNK, BCG * CHUNK], F32, tag=f"bc{j}",
                           name=f"bc{j}")
                  for j in range(NBC)]

            def bc_mm(j):
                lo = RR_R + j * BCG * CHUNK
                nc.tensor.matmul(
                    bc[j][:],
                    lhsT=ones_v,
                    rhs=rr_sb[:, lo:lo + BCG * CHUNK],
                    start=True, stop=True,
                )

            def rview(i):
                j, c = divmod(i, BCG)
                return bc[j][:, c * CHUNK:(c + 1) * CHUNK]

            bc_mm(0)

            yout = y.rearrange("(i p r) d -> i p r d", p=CHUNK, r=rep)
            for i in range(C):
                # Stagger the remaining R broadcasts so each lands ~2
                # chunks before its first reader.
                if i in (2, 6, 10):
                    bc_mm(i // 4 + 1)
                nb = nbands[i]
                ps = psp.tile([CHUNK, QW], F32, tag="ps")
                for idx, bandk in enumerate(range(nb - 1, -1, -1)):
                    lt_t = ltp.tile([CHUNK, CHUNK], mm_dt, tag="lt")
                    bcol = CT_BIAS + bandk * C + i
                    bias = ct_sb[:, bcol:bcol + 1]
                    if bandk == 0:
                        # arg = (R + bias) + mneg fused on DVE, then plain exp
                        arg = argp.tile([CHUNK, CHUNK], F32, tag="arg")
                        nc.vector.scalar_tensor_tensor(
                            arg[:], rview(i), bias, mneg_v,
                            op0=mybir.AluOpType.add, op1=mybir.AluOpType.add,
                        )
                        nc.scalar.activation(
                            lt_t[:], arg[:], mybir.ActivationFunctionType.Exp)
                    else:
                        nc.scalar.activation(
                            lt_t[:], rview(i), mybir.ActivationFunctionType.Exp,
                            bias=bias)
                    nc.tensor.matmul(
                        ps[:],
                        lhsT=lt_t[:],
                        rhs=xview(i - bandk),
                        start=(idx == 0), stop=(idx == nb - 1),
                    )
                # PSUM -> bf16 staging, split across the two copy engines
                yt = yp.tile([CHUNK, QW], mm_dt, tag="yb")
                if i % 2 == 0:
                    nc.vector.tensor_copy(yt[:], ps[:])
                else:
                    nc.scalar.copy(yt[:], ps[:])
                # One DMA writes both rep copies: the SBUF side reads
                # each row twice via a stride-0 broadcast dim.
                src = yt[:].unsqueeze(1).broadcast_to([CHUNK, rep, QW])
                eng = nc.sync if i % 2 == 0 else nc.gpsimd
                eng.dma_start(out=yout[i], in_=src)
    nc.compile()
    return nc


def _run(inputs, trace=False):
    hidden = np.asarray(inputs["hidden_states"], dtype=np.float32)
    logw, cumA, plug = _host_precompute(inputs["boundary_mask"],
                                        inputs["boundary_prob"])

    rep = LFULL // M
    fast = np.array_equal(
        plug, np.tile(np.repeat(np.arange(M), rep)[None, :], (plug.shape[0], 1))
    )
    if not fast:
        return _numpy_fallback(hidden, logw, cumA, plug), None

    nbands = _decide_bands(cumA, logw)
    key = (nbands, rep, USE_BF16)
    if key not in _prog_cache:
        _prog_cache[key] = _build_program(nbands, rep, USE_BF16)
    nc = _prog_cache[key]

    # Host-side per-core inputs.
    maxband = max(nbands)
    ct_w = CHUNK + maxband * C
    rrow_np = np.empty((B, 1, CHUNK + C * CHUNK), np.float32)
    rrow_np[:, :, RR_ONES:RR_ONES + CHUNK] = 1.0
    ct_np = np.zeros((B, CHUNK, ct_w), np.float32)
    ct_np[:, :, CT_MNEG:CT_MNEG + CHUNK] = np.where(
        np.arange(CHUNK)[:, None] > np.arange(CHUNK)[None, :],
        np.float32(MNEG), np.float32(0.0),
    )[None]
    for b in range(B):
        for i in range(C):
            T0 = i * CHUNK
            rrow_np[b, 0, RR_R + i * CHUNK:RR_R + (i + 1) * CHUNK] = (
                cumA[b, T0:T0 + CHUNK] - cumA[b, T0]
            ).astype(np.float32)
            for k in range(nbands[i]):
                S0 = (i - k) * CHUNK
                ct_np[b, :, CT_BIAS + k * C + i] = (
                    cumA[b, T0] - cumA[b, S0:S0 + CHUNK] + logw[b, S0:S0 + CHUNK]
                ).astype(np.float32)

    import ml_dtypes
    x_dt = ml_dtypes.bfloat16 if USE_BF16 else np.float32
    in_maps = []
    for c in range(NCORES):
        b, q = divmod(c, NQ)
        in_maps.append({
            "x": np.ascontiguousarray(hidden[b, :, q * QW:(q + 1) * QW]).astype(x_dt),
            "rrow": rrow_np[b],
            "ct": ct_np[b],
        })

    res = run_bass_kernel_spmd(nc, in_maps, list(range(NCORES)), trace=trace)
    out = np.empty((B, LFULL, D_MODEL), np.float32)
    for c in range(NCORES):
        b, q = divmod(c, NQ)
        out[b, :, q * QW:(q + 1) * QW] = res.results[c]["y"].astype(np.float32)
    return out, res


def _numpy_fallback(hidden, logw, cumA, plug):
    """Exact CPU path for plug patterns the device program doesn't cover."""
    y = np.zeros((B, M, D_MODEL), np.float32)
    for b in range(B):
        for i in range(C):
            T0 = i * CHUNK
            acc = np.zeros((CHUNK, D_MODEL), np.float64)
            for j in range(i + 1):
                S0 = j * CHUNK
                arg = (cumA[b, T0:T0 + CHUNK][None, :]
                       - cumA[b, S0:S0 + CHUNK][:, None]
                       + logw[b, S0:S0 + CHUNK][:, None])
                if j == i:
                    s_idx = np.arange(CHUNK)
                    arg = np.where(s_idx[:, None] > s_idx[None, :], -np.inf, arg)
                if arg.max() < UFLOW:
                    continue
                LT = np.exp(arg)
                acc += LT.T @ hidden[b, S0:S0 + CHUNK].astype(np.float64)
            y[b, T0:T0 + CHUNK] = acc.astype(np.float32)
    return np.take_along_axis(y, plug[:, :, None].astype(np.int64), axis=1)


def kernel(**inputs) -> np.ndarray:
    out, _ = _run(inputs, trace=False)
    return out

